# revision 53
# baseline (speedup 1.0000x reference)
"""MLA attention kernel for Trainium2 — 8-core tensor-parallel (self-contained).

Sharding: data-parallel over batch (2) x tensor-parallel over head groups
(4 groups of 4 heads) = 8 cores, SPMD (one NEFF, per-core input shards).
Core ci: batch ci//4, heads [4*(ci%4), 4*(ci%4)+4).

Layout highlights:
  - every weight is host-pre-reshaped into its SBUF-resident partition-major
    layout so it loads as ONE large DMA (amortizes the ~2us DMA fixed cost);
    wqd is blocked by rank-group so q-down consumes blocks as they land
  - all intermediates (k_nope / v / q_nope / q_rope / attention out) stay in
    SBUF between phases; only the latent gathers bounce through DRAM
  - collective buffers are partition-major, so packs/readbacks are single
    transfers; the k-rope row-duplication needed for PE row-group packing is
    baked into the gathered block
  - softmax denominators use an all-ones [128,128] stationary matmul per
    chunk-pair (broadcast into all 128 partitions -> reciprocal runs on all
    DVE lanes); the causal mask is ADDED to score PSUM via an identity
    matmul before exp, keeping the DVE off the critical path
  - score PSUM tiles are [128,1024] (2 banks): one ACT exp instruction
    covers two key chunks; the two K=64 rope matmuls of a chunk pair run
    concurrently in disjoint PE row-groups (partitions 0-63 / 64-127)
  - out-projection is interleaved into the attention loop per query chunk,
    output written fp16
"""

import math

import numpy as np
import ml_dtypes

# ---- problem constants (from the reference model) ----
B, S, HID = 2, 2048, 2048
H, D_NOPE, D_ROPE, V_DIM = 16, 128, 64, 128
KV_RANK, Q_RANK = 512, 1536
HEAD_DIM = D_NOPE + D_ROPE
THETA, EPS = 10000.0, 1e-6
NCORES = 8
NH = 4                    # heads per core
T = 512                   # token chunk
NT = S // T
QC = 512                  # attention query chunk
NQC = S // QC
KH = HID // 128           # 16 k-chunks over HID
RQ = Q_RANK // 128        # 12 chunks over q rank
RKV = KV_RANK // 128      # 4 chunks over kv rank
SCALE = 1.0 / math.sqrt(HEAD_DIM)

_CACHE = {}


def build_nc():
    """Build the Bass/Tile program (one NeuronCore, run SPMD on 8)."""
    from contextlib import ExitStack

    import concourse.mybir as mybir
    import concourse.tile as tile
    from concourse import bacc
    from concourse.bass import ds

    dt = mybir.dt
    AF = mybir.ActivationFunctionType
    bf16 = dt.bfloat16
    f32 = dt.float32
    f16 = dt.float16

    nc = bacc.Bacc(
        "TRN2",
        target_bir_lowering=False,
        debug=False,
        enable_asserts=False,
        num_devices=NCORES,
    )

    TL = S // 4  # local token quarter

    # ---- I/O (all partition-major, host pre-reshaped) ----
    x_ap = nc.dram_tensor("x", [128, KH, TL], bf16, kind="ExternalInput").ap()
    wqd_ap = nc.dram_tensor("wqd", [128, 3, KH, 512], bf16, kind="ExternalInput").ap()
    wqu_ap = nc.dram_tensor("wqu", [128, RQ, NH * HEAD_DIM], bf16, kind="ExternalInput").ap()
    wkvd_ap = nc.dram_tensor("wkvd", [128, KH, KV_RANK + D_ROPE], bf16, kind="ExternalInput").ap()
    wkvuk_ap = nc.dram_tensor("wkvuk", [128, RKV, NH * D_NOPE], bf16, kind="ExternalInput").ap()
    wkvuv_ap = nc.dram_tensor("wkvuv", [128, RKV, NH * V_DIM], bf16, kind="ExternalInput").ap()
    wout_ap = nc.dram_tensor("wout", [128, NH, HID], bf16, kind="ExternalInput").ap()
    cos_ap = nc.dram_tensor("cosq", [128, S], bf16, kind="ExternalInput").ap()
    sin_ap = nc.dram_tensor("sinq", [128, S], bf16, kind="ExternalInput").ap()
    mask_ap = nc.dram_tensor("maskp", [128, 2, 1024], bf16, kind="ExternalInput").ap()
    ones_ap = nc.dram_tensor("ones128", [128, 128], bf16, kind="ExternalInput").ap()
    ident_ap = nc.dram_tensor("ident128", [128, 128], bf16, kind="ExternalInput").ap()
    cosl_ap = nc.dram_tensor("cosl", [128, TL], bf16, kind="ExternalInput").ap()
    sinl_ap = nc.dram_tensor("sinl", [128, TL], bf16, kind="ExternalInput").ap()
    out_ap = nc.dram_tensor("out", [S, HID], f16, kind="ExternalOutput").ap()

    with tile.TileContext(nc) as tc, ExitStack() as ctx:
        # ---- PSUM pools: 2x[128,1024] + 2x[128,512] + 2x[128,512] = 8 banks
        sc_ps = ctx.enter_context(tc.tile_pool(name="sc_ps", bufs=2, space="PSUM"))
        pv_ps = ctx.enter_context(tc.tile_pool(name="pv_ps", bufs=2, space="PSUM"))
        aux_ps = ctx.enter_context(tc.tile_pool(name="aux_ps", bufs=2, space="PSUM"))

        def sc_half(state, idx):
            # rotate [128,1024] sc tiles, handing out 512-wide halves
            if idx % 2 == 0:
                state["t"] = sc_ps.tile([128, 1024], f32, tag="sc", name="sct")
            return state["t"][:, ds((idx % 2) * 512, 512)]

        const = ctx.enter_context(tc.tile_pool(name="const", bufs=1))
        woutp = ctx.enter_context(tc.tile_pool(name="woutp", bufs=1))
        dram = ctx.enter_context(tc.tile_pool(name="dram", bufs=1, space="DRAM"))

        # ---- up-projection weights (outlive w1 -> allocated below it)
        w2 = tc.alloc_tile_pool(name="w2", bufs=1)
        wkvuk_sb = w2.tile([128, RKV, NH * D_NOPE], bf16, tag="wkvuk")
        wkvuv_sb = w2.tile([128, RKV, NH * V_DIM], bf16, tag="wkvuv")
        wqu_sb = w2.tile([128, RQ, NH * HEAD_DIM], bf16, tag="wqu")

        # ---- phase-A0 weights + x (released after A0)
        w1 = tc.alloc_tile_pool(name="w1", bufs=1)
        xt = w1.tile([128, KH, TL], bf16, tag="xt")
        wkvd_sb = w1.tile([128, KH, KV_RANK + D_ROPE], bf16, tag="wkvd")
        wqd_sb = w1.tile([128, 3, KH, 512], bf16, tag="wqd")

        # priority-ordered big transfers: xt on sync, wkvd on the (idle)
        # scalar queue so kv-down's operands land in parallel and wqd moves
        # up the sync stream.
        for q4 in range(2):
            hk = ds(q4 * (KH // 2), KH // 2)
            nc.sync.dma_start(out=xt[:, hk, :], in_=x_ap[:, hk, :])
            nc.scalar.dma_start(out=wkvd_sb[:, hk, :], in_=wkvd_ap[:, hk, :])
        for g in range(3):
            nc.sync.dma_start(out=wqd_sb[:, g, :, :], in_=wqd_ap[:, g, :, :])
        nc.sync.dma_start(out=wkvuk_sb[:], in_=wkvuk_ap[:])
        nc.sync.dma_start(out=wkvuv_sb[:], in_=wkvuv_ap[:])

        # ---- resident constants
        cos_sb = const.tile([128, S], bf16, name="cos_sb")
        nc.sync.dma_start(out=cos_sb[:], in_=cos_ap[:])
        sin_sb = const.tile([128, S], bf16, name="sin_sb")
        nc.sync.dma_start(out=sin_sb[:], in_=sin_ap[:])
        cosl_sb = const.tile([128, TL], bf16, name="cosl_sb")
        nc.sync.dma_start(out=cosl_sb[:], in_=cosl_ap[:])
        sinl_sb = const.tile([128, TL], bf16, name="sinl_sb")
        nc.sync.dma_start(out=sinl_sb[:], in_=sinl_ap[:])
        mask_sb = const.tile([128, 2, 1024], bf16, name="mask_sb")
        nc.sync.dma_start(out=mask_sb[:], in_=mask_ap[:])
        ones_sb = const.tile([128, 128], bf16, name="ones_sb")
        nc.sync.dma_start(out=ones_sb[:], in_=ones_ap[:])
        ident_sb = const.tile([128, 128], bf16, name="ident_sb")
        nc.sync.dma_start(out=ident_sb[:], in_=ident_ap[:])
        wout_sb = woutp.tile([128, NH, HID], bf16, tag="wout")

        krope2_sb = const.tile([128, S], bf16, name="krope2_sb")
        at_sb = const.tile([128, NH, S], bf16, name="at_sb")
        eps_sb = const.tile([128, 1], f32, name="eps_sb")
        nc.gpsimd.memset(eps_sb[:], EPS)

        # DRAM bounce buffers for the latent gathers (partition-major; the
        # kv block 4 carries the row-duplicated rotated k-rope)
        gin_kv = dram.tile([128, RKV + 1, TL], bf16, name="gin_kv")
        gout_kv = dram.tile([4, 128, RKV + 1, TL], bf16, name="gout_kv")
        gin_q = [dram.tile([128, 6, TL], bf16, name=f"gin_q{g}") for g in range(2)]
        gout_q = [dram.tile([4, 128, 6, TL], bf16, name=f"gout_q{g}") for g in range(2)]
        GROUPS = [[0, 1, 2, 3], [4, 5, 6, 7]]

        # ================= phase A0: local down-projections =================
        # k-outer: each weight chunk is consumed as soon as its DMA lands.
        wa = tc.alloc_tile_pool(name="wa", bufs=2)
        st = {}
        kvc_bf = wa.tile([128, RKV, TL], bf16, tag="kvc", bufs=1)
        sq_bf = wa.tile([128, RKV, TL], bf16, tag="sq", bufs=1)
        ms_ps = aux_ps.tile([128, TL], f32, tag="aux", name="ms_ps")
        krp_ps = aux_ps.tile([128, TL], f32, tag="aux", name="krp_ps")
        kv_ps = [sc_half(st, j) for j in range(RKV)]
        for k in range(KH):
            for j in range(RKV):
                nc.tensor.matmul(
                    kv_ps[j], wkvd_sb[:, k, ds(j * 128, 128)], xt[:, k, :],
                    start=(k == 0), stop=(k == KH - 1),
                )
            nc.tensor.matmul(
                krp_ps[0:64, :], wkvd_sb[:, k, ds(KV_RANK, D_ROPE)], xt[:, k, :],
                start=(k == 0), stop=(k == KH - 1),
            )
        for j in range(RKV):
            # square on DVE (from the SBUF copy): keeps ACT clear — no
            # Square->Sqrt table switch on the collective-gating rmsnorm chain
            nc.vector.tensor_copy(kvc_bf[:, j, :], kv_ps[j])
            nc.vector.tensor_mul(sq_bf[:, j, :], kvc_bf[:, j, :], kvc_bf[:, j, :])
        for j in range(RKV):
            nc.tensor.matmul(
                ms_ps, ones_sb[:], sq_bf[:, j, :],
                start=(j == 0), stop=(j == RKV - 1),
            )
        # rinv = 1/sqrt(ms/512 + eps), already broadcast across partitions
        srt = wa.tile([128, TL], f32, tag="srt", bufs=1)
        nc.scalar.activation(srt, ms_ps, AF.Sqrt, bias=eps_sb[:], scale=1.0 / KV_RANK)
        rinv = wa.tile([128, TL], f32, tag="rinv", bufs=1)
        nc.vector.reciprocal_approx_fast(out=rinv, in_=srt)
        kvcn = wa.tile([128, RKV, TL], bf16, tag="kvcn", bufs=1)
        for j in range(RKV):
            nc.vector.tensor_mul(kvcn[:, j, :], kvc_bf[:, j, :], rinv)
        nc.scalar.dma_start(out=gin_kv[:, 0:RKV, :], in_=kvcn[:])
        # k rope rotate (local quarter, local cos/sin), duplicated to 64-127
        kr_raw = wa.tile([64, TL], f32, tag="kr_raw", bufs=1)
        nc.vector.tensor_copy(kr_raw, krp_ps[0:64, :])
        kr_sh = wa.tile([64, TL], f32, tag="kr_sh", bufs=1)
        nc.scalar.dma_start(out=kr_sh[0:32, :], in_=kr_raw[32:64, :])
        nc.scalar.dma_start(out=kr_sh[32:64, :], in_=kr_raw[0:32, :])
        kt1 = wa.tile([64, TL], f32, tag="kt1", bufs=1)
        kt2 = wa.tile([64, TL], f32, tag="kt2", bufs=1)
        nc.vector.tensor_mul(kt1, kr_raw, cosl_sb[0:64, :])
        nc.vector.tensor_mul(kt2, kr_sh, sinl_sb[0:64, :])
        krl2 = wa.tile([128, TL], bf16, tag="krl2", bufs=1)
        nc.vector.tensor_sub(krl2[0:32, :], kt1[0:32, :], kt2[0:32, :])
        nc.vector.tensor_add(krl2[32:64, :], kt1[32:64, :], kt2[32:64, :])
        nc.scalar.dma_start(out=krl2[64:128, :], in_=krl2[0:64, :])
        nc.scalar.dma_start(out=gin_kv[:, RKV, :], in_=krl2[:])
        nc.gpsimd.collective_compute(
            "AllGather", mybir.AluOpType.bypass, replica_groups=GROUPS,
            ins=[gin_kv.opt()], outs=[gout_kv.opt()],
        )
        # late-needed weights at the tail of the sync stream
        nc.sync.dma_start(out=wqu_sb[:], in_=wqu_ap[:])
        nc.sync.dma_start(out=wout_sb[:], in_=wout_ap[:])

        # ---- q down: k-outer over compute-groups of 4 (PSUM-limited), but
        # gathered in two rank-halves of 6 (fewer setups on the serial
        # collective stream -> the last gather lands earlier) ----
        qlat = wa.tile([128, RQ, TL], bf16, tag="qlat", bufs=1)
        for g in range(3):
            qps = [sc_half(st, m) for m in range(4)]
            for k in range(KH):
                for m in range(4):
                    nc.tensor.matmul(
                        qps[m], wqd_sb[:, g, k, ds(m * 128, 128)], xt[:, k, :],
                        start=(k == 0), stop=(k == KH - 1),
                    )
            for m in range(4):
                gm = 4 * g + m
                nc.vector.tensor_copy(qlat[:, gm, :], qps[m])
            if g == 0:
                nc.scalar.dma_start(out=gin_q[0][:, 0:4, :], in_=qlat[:, 0:4, :])
            elif g == 1:
                nc.scalar.dma_start(out=gin_q[0][:, 4:6, :], in_=qlat[:, 4:6, :])
                nc.gpsimd.collective_compute(
                    "AllGather", mybir.AluOpType.bypass, replica_groups=GROUPS,
                    ins=[gin_q[0].opt()], outs=[gout_q[0].opt()],
                )
                nc.scalar.dma_start(out=gin_q[1][:, 0:2, :], in_=qlat[:, 6:8, :])
            else:
                nc.scalar.dma_start(out=gin_q[1][:, 2:6, :], in_=qlat[:, 8:12, :])
                nc.gpsimd.collective_compute(
                    "AllGather", mybir.AluOpType.bypass, replica_groups=GROUPS,
                    ins=[gin_q[1].opt()], outs=[gout_q[1].opt()],
                )

        wa.release()
        w1.release()
        # SBUF-resident intermediates (reuse w1's region)
        kvsb = tc.alloc_tile_pool(name="kvsb", bufs=1)
        kn_sb = kvsb.tile([128, NH, S], bf16, tag="kn")
        v_sb = kvsb.tile([128, S // 128, NH * V_DIM], bf16, tag="v")
        qn_sb = kvsb.tile([128, NH, S], bf16, tag="qn")
        qr2_sb = kvsb.tile([128, NH, S], bf16, tag="qr2")
        wb = tc.alloc_tile_pool(name="wb", bufs=2)

        # ================= phase A1: kv up-projections per chunk =================
        for c in range(NT):
            csl = ds(c * T, T)
            kvg = wb.tile([128, RKV, T], bf16, tag="kvg", bufs=2)
            nc.scalar.dma_start(out=kvg[:], in_=gout_kv[c, :, 0:RKV, :])
            for m in range(NH):
                ps = sc_half(st, m)
                for j in range(RKV):
                    nc.tensor.matmul(
                        ps, wkvuk_sb[:, j, ds(m * 128, 128)], kvg[:, j, :],
                        start=(j == 0), stop=(j == RKV - 1),
                    )
                nc.vector.tensor_copy(kn_sb[:, m, csl], ps)
            for s2 in range(T // 128):
                ps = sc_half(st, s2)
                for j in range(RKV):
                    nc.tensor.matmul(
                        ps, kvg[:, j, ds(s2 * 128, 128)], wkvuv_sb[:, j, :],
                        start=(j == 0), stop=(j == RKV - 1),
                    )
                nc.vector.tensor_copy(v_sb[:, c * (T // 128) + s2, :], ps)

        # krope full (block 4 of the kv gather, already row-duplicated);
        # issued after A1's readbacks — it is only needed in phase B
        for c in range(NT):
            nc.scalar.dma_start(
                out=krope2_sb[:, ds(c * TL, TL)], in_=gout_kv[c, :, RKV, :]
            )

        # ================= phase A2: q up-projections per chunk =================
        # r-grouped: rank-group g's matmuls only need the g-th q gather.
        for c in range(NT):
            csl = ds(c * T, T)
            qlg = wb.tile([128, RQ, T], bf16, tag="qlg", bufs=2)
            qn_ps = [sc_half(st, m) for m in range(NH)]
            ps1 = pv_ps.tile([128, T], f32, tag="pv", name="rp1")
            ps2 = pv_ps.tile([128, T], f32, tag="pv", name="rp2")
            for g in range(2):
                nc.scalar.dma_start(out=qlg[:, ds(6 * g, 6), :], in_=gout_q[g][c])
            for r in range(RQ):
                for m in range(NH):
                    nc.tensor.matmul(
                        qn_ps[m], wqu_sb[:, r, ds(m * 128, 128)], qlg[:, r, :],
                        start=(r == 0), stop=(r == RQ - 1),
                    )
                nc.tensor.matmul(
                    ps1, wqu_sb[:, r, ds(NH * D_NOPE, 128)], qlg[:, r, :],
                    start=(r == 0), stop=(r == RQ - 1),
                )
                nc.tensor.matmul(
                    ps2, wqu_sb[:, r, ds(NH * D_NOPE + 128, 128)], qlg[:, r, :],
                    start=(r == 0), stop=(r == RQ - 1),
                )
            for m in range(NH):
                nc.vector.tensor_copy(qn_sb[:, m, csl], qn_ps[m])
            qa = wb.tile([128, T], f32, tag="qa", bufs=1)
            qb = wb.tile([128, T], f32, tag="qb", bufs=1)
            nc.vector.tensor_mul(qa, ps1, cos_sb[:, csl])
            nc.vector.tensor_mul(qb, ps2, sin_sb[:, csl])
            y1 = wb.tile([128, T], bf16, tag="y1", bufs=2)
            nc.vector.tensor_sub(y1, qa, qb)
            qa2 = wb.tile([128, T], f32, tag="qa", bufs=1)
            qb2 = wb.tile([128, T], f32, tag="qb", bufs=1)
            nc.vector.tensor_mul(qa2, ps2, cos_sb[:, csl])
            nc.vector.tensor_mul(qb2, ps1, sin_sb[:, csl])
            y2 = wb.tile([128, T], bf16, tag="y2", bufs=2)
            nc.vector.tensor_add(y2, qa2, qb2)
            # assemble per-head [x1(32); x2(32)] rope layout, duplicated at
            # 64-127 (sync queue, idle by now)
            for h in range(NH):
                nc.sync.dma_start(out=qr2_sb[0:32, h, csl], in_=y1[ds(32 * h, 32), :])
                nc.sync.dma_start(out=qr2_sb[32:64, h, csl], in_=y2[ds(32 * h, 32), :])
                nc.sync.dma_start(out=qr2_sb[64:96, h, csl], in_=y1[ds(32 * h, 32), :])
                nc.sync.dma_start(out=qr2_sb[96:128, h, csl], in_=y2[ds(32 * h, 32), :])

        # ================= phase B + C: attention with interleaved out-proj =====
        wb.release()
        wc = tc.alloc_tile_pool(name="wc", bufs=2)

        norm_pend = []

        def drain_norm(stn):
            h_, qsl_, pv_, den_ = stn
            rec = wc.tile([128, QC], f32, tag="rec", bufs=2)
            nc.vector.reciprocal_approx_fast(out=rec, in_=den_)
            nc.vector.tensor_mul(at_sb[:, h_, qsl_], pv_, rec)

        for qc in range(NQC):
            qsl = ds(qc * QC, QC)
            nkc = 4 * qc + 4
            npair = nkc // 2
            for h in range(NH):
                pv = pv_ps.tile([128, QC], f32, tag="pv")
                den_ps = aux_ps.tile([128, QC], f32, tag="aux", name="den_ps")
                pend = []
                for t in range(npair):
                    kcA, kcB = 2 * t, 2 * t + 1
                    dA, dB = kcA - 4 * qc, kcB - 4 * qc
                    sct = sc_ps.tile([128, 1024], f32, tag="sc", name="sct_b")
                    nc.tensor.matmul(
                        sct[:, 0:512], kn_sb[:, h, ds(kcA * 128, 128)], qn_sb[:, h, qsl],
                        start=True, stop=False,
                    )
                    nc.tensor.matmul(
                        sct[:, 512:1024], kn_sb[:, h, ds(kcB * 128, 128)], qn_sb[:, h, qsl],
                        start=True, stop=False,
                    )
                    # the two K=64 rope matmuls land in disjoint row-groups -> concurrent
                    nc.tensor.matmul(
                        sct[:, 0:512], krope2_sb[0:64, ds(kcA * 128, 128)],
                        qr2_sb[0:64, h, qsl], start=False, stop=(dA < 0),
                    )
                    nc.tensor.matmul(
                        sct[:, 512:1024], krope2_sb[64:128, ds(kcB * 128, 128)],
                        qr2_sb[64:128, h, qsl], start=False, stop=(dB < 0),
                    )
                    if dA >= 0:
                        # causal mask: add -3e4 on invalid entries via identity matmul
                        pi = dA // 2
                        nc.tensor.matmul(
                            sct[:, 0:512], ident_sb[:], mask_sb[:, pi, 0:512],
                            start=False, stop=True,
                        )
                        nc.tensor.matmul(
                            sct[:, 512:1024], ident_sb[:], mask_sb[:, pi, 512:1024],
                            start=False, stop=True,
                        )
                    E = wc.tile([128, 1024], bf16, tag="E", bufs=6)
                    nc.scalar.activation(E, sct, AF.Exp, scale=SCALE)
                    Eh = wc.tile([128, QC], bf16, tag="Eh", bufs=4)
                    nc.vector.tensor_add(Eh, E[:, 0:512], E[:, 512:1024])
                    pend.append((t, E, Eh))
                    if len(pend) > (2 if npair >= 3 else 1):
                        pt, pE, pEh = pend.pop(0)
                        nc.tensor.matmul(
                            pv, v_sb[:, 2 * pt, ds(h * V_DIM, V_DIM)], pE[:, 0:512],
                            start=(pt == 0), stop=False,
                        )
                        nc.tensor.matmul(
                            pv, v_sb[:, 2 * pt + 1, ds(h * V_DIM, V_DIM)], pE[:, 512:1024],
                            start=False, stop=False,
                        )
                        nc.tensor.matmul(
                            den_ps, ones_sb[:], pEh,
                            start=(pt == 0), stop=False,
                        )
                while pend:
                    pt, pE, pEh = pend.pop(0)
                    last = not pend
                    nc.tensor.matmul(
                        pv, v_sb[:, 2 * pt, ds(h * V_DIM, V_DIM)], pE[:, 0:512],
                        start=(pt == 0), stop=False,
                    )
                    nc.tensor.matmul(
                        pv, v_sb[:, 2 * pt + 1, ds(h * V_DIM, V_DIM)], pE[:, 512:1024],
                        start=False, stop=last,
                    )
                    nc.tensor.matmul(
                        den_ps, ones_sb[:], pEh,
                        start=(pt == 0), stop=last,
                    )
                norm_pend.append((h, qsl, pv, den_ps))
                if len(norm_pend) > 1:
                    drain_norm(norm_pend.pop(0))
            while norm_pend:
                drain_norm(norm_pend.pop(0))
            # ---- out-projection for this qc's 4 token blocks ----
            for t16 in range(qc * 4, qc * 4 + 4):
                o_row = wc.tile([128, HID], f16, tag="ot", bufs=2)
                for n in range(HID // 512):
                    # rotate across sc halves AND pv tiles for a 6-deep psum
                    # rotation (hides the cast WAR)
                    if n < 2:
                        ps = sc_half(st, n)
                    else:
                        ps = pv_ps.tile([128, 512], f32, tag="pv", name="cps")
                    for f in range(NH):
                        nc.tensor.matmul(
                            ps, at_sb[:, f, ds(t16 * 128, 128)], wout_sb[:, f, ds(n * 512, 512)],
                            start=(f == 0), stop=(f == NH - 1),
                        )
                    nc.vector.tensor_copy(o_row[:, ds(n * 512, 512)], ps)
                nc.sync.dma_start(out=out_ap[ds(t16 * 128, 128), :], in_=o_row)

        wc.release()
        kvsb.release()
        w2.release()

    nc.compile()
    return nc


def get_nc():
    if "nc" not in _CACHE:
        _CACHE["nc"] = build_nc()
    return _CACHE["nc"]


def host_inputs(x, w_q_down, w_q_up, w_kv_down, kv_norm_w, w_kv_up, w_out):
    """Build the 8 per-core input shards (host-side prep, numpy only)."""
    bf = ml_dtypes.bfloat16
    x = np.asarray(x, np.float32)
    inv = 1.0 / THETA ** (np.arange(0, D_ROPE, 2, dtype=np.float64) / D_ROPE)
    ang = np.arange(S, dtype=np.float64)[:, None] * inv[None, :]      # (S, 32)
    cosq = np.ascontiguousarray(np.tile(np.cos(ang).T, (4, 1))).astype(bf)  # (128, S)
    sinq = np.ascontiguousarray(np.tile(np.sin(ang).T, (4, 1))).astype(bf)
    # additive causal masks for the diagonal-band chunks, paired (d, d+1)
    maskp = np.zeros((128, 2, 1024), np.float32)
    r = np.arange(128)[:, None]
    j = np.arange(512)[None, :]
    for d in range(4):
        maskp[:, d // 2, 512 * (d % 2) : 512 * (d % 2) + 512] = np.where(
            j >= 128 * d + r, 0.0, -30000.0
        )
    maskp = maskp.astype(bf)
    ones128 = np.ones((128, 128), bf)
    ident128 = np.eye(128, dtype=np.float32).astype(bf)
    wkv_eff = np.asarray(w_kv_up, np.float32) * np.asarray(kv_norm_w, np.float32)[:, None]

    def pmaj(w, *shape):
        # [K*128, N] -> partition-major [128, K, N] (-> optional extra reshape)
        kk = w.shape[0] // 128
        out = np.ascontiguousarray(w.reshape(kk, 128, w.shape[1]).transpose(1, 0, 2))
        return out.reshape(shape) if shape else out

    xT_bf = [np.ascontiguousarray(x[b].T).astype(bf) for b in range(B)]
    wqd_bf = np.asarray(w_q_down, np.float32).astype(bf)
    # wqd: [128, 3 rank-groups, 16 k-chunks, 512]
    wqd_pm = np.ascontiguousarray(
        wqd_bf.reshape(KH, 128, 3, 512).transpose(1, 2, 0, 3)
    )
    wkvd_pm = pmaj(np.asarray(w_kv_down, np.float32).astype(bf))
    wqu_f = np.asarray(w_q_up, np.float32)
    wout_f = np.asarray(w_out, np.float32)

    in_maps = []
    for ci in range(NCORES):
        b, hg = divmod(ci, 4)
        heads = list(range(NH * hg, NH * hg + NH))
        qu_cols = (
            [h * HEAD_DIM + j2 for h in heads for j2 in range(D_NOPE)]
            + [h * HEAD_DIM + D_NOPE + j2 for h in heads for j2 in range(32)]
            + [h * HEAD_DIM + D_NOPE + 32 + j2 for h in heads for j2 in range(32)]
        )
        kn_cols = [h * (D_NOPE + V_DIM) + j2 for h in heads for j2 in range(D_NOPE)]
        v_cols = [h * (D_NOPE + V_DIM) + D_NOPE + j2 for h in heads for j2 in range(V_DIM)]
        xq = np.ascontiguousarray(xT_bf[b][:, 512 * hg : 512 * (hg + 1)])
        in_maps.append(
            {
                "x": pmaj(xq),
                "cosl": np.ascontiguousarray(cosq[:, 512 * hg : 512 * (hg + 1)]),
                "sinl": np.ascontiguousarray(sinq[:, 512 * hg : 512 * (hg + 1)]),
                "wqd": wqd_pm,
                "wqu": pmaj(np.ascontiguousarray(wqu_f[:, qu_cols]).astype(bf)),
                "wkvd": wkvd_pm,
                "wkvuk": pmaj(np.ascontiguousarray(wkv_eff[:, kn_cols]).astype(bf)),
                "wkvuv": pmaj(np.ascontiguousarray(wkv_eff[:, v_cols]).astype(bf)),
                "wout": pmaj(
                    np.ascontiguousarray(
                        wout_f[NH * V_DIM * hg : NH * V_DIM * (hg + 1), :]
                    ).astype(bf)
                ),
                "cosq": cosq,
                "sinq": sinq,
                "maskp": maskp,
                "ones128": ones128,
                "ident128": ident128,
            }
        )
    return in_maps


def run(inputs, trace=False, trace_cores=None):
    from concourse.bass_utils import run_bass_kernel_spmd

    nc = get_nc()
    in_maps = host_inputs(**inputs)
    res = run_bass_kernel_spmd(
        nc,
        in_maps,
        core_ids=list(range(NCORES)),
        trace=trace,
        trace_cores=trace_cores,
    )
    out = np.zeros((B, S, HID), np.float32)
    for ci in range(NCORES):
        out[ci // 4] += res.results[ci]["out"].astype(np.float32)
    return out, res


def kernel(**inputs):
    out, _ = run(inputs, trace=False)
    return out
